# revision 10
# baseline (speedup 1.0000x reference)
"""DecoderRNN (LSTM + vocab projection + log_softmax) on 8 Trainium2 cores.

Strategy (self-contained; B=64, T=50, H=1024, V=32000 hardcoded):
  - Host: token shift, embedding gather + relu, transposes/casts to fp16,
    per-core weight slicing.
  - Phase A: x_gates = W_ih_k @ x^T + b_k (gate-dim sharded), staged to DRAM.
  - Phase B: LSTM recurrence, tensor-parallel over the hidden dim; core k
    owns h/c slice [128]. Per step: gates^T = W_hh_k @ h^T (fp16 operands,
    f32 psum), nonlinearities, AllGather of h_k^T across 8 cores (50 AGs).
  - Phase C (interleaved into B's AllGather latency): logits for the core's
    4000-vocab slice in [128bt, 4000] tiles held fp16 in an SBUF ring;
    sum-exp via ACT accum_out (logits are O(1): no max subtraction needed);
    per-tile sums piggybacked on the phase-B AllGathers; logp written
    directly from the ring once the global sum arrives.
  - Host: concat vocab slices; assemble hT/cT from per-core slices.
"""
import numpy as np

B, T, H, V = 64, 50, 1024, 32000
NC = 8
HPC = H // NC          # 128
VPC = V // NC          # 4000
KT = H // 128          # 8
NT = (B * T) // 128    # 25 bt tiles (t-major: col = t*64 + b)
NCH = 8                # C1 psum chunks per bt tile (500 wide)
VCH = VPC // NCH       # 500
C2N = 4                # C2 chunks per tile
C2W = VPC // C2N       # 1000
SOS = 0

_compiled = None


def _pacing():
    """Unified schedule. Returns per-step dict with:
       pe: list of ("A", chunk, m) | ("C1", j, n) work items
       attach: tile j whose sumexp rides this step's AllGather (or None)
       c2: list of (j, n) logp-write chunks
    plus upfront A pairs and tail lists. A chunks are 256 bt-cols wide.
    """
    NAC = 13                     # A chunks: 12*256 + 1*128
    a_q = [("A", c, m) for c in range(2, NAC) for m in range(4)]
    plan = {t: {"pe": [], "attach": None, "c2": []} for t in range(T)}
    tail = {"c1": [], "corr": [], "c2": []}
    c1_q = []
    nxt = 0
    done_step = {}
    for t in range(T):
        while nxt < NT and t >= 2 * nxt + 3:
            c1_q.extend(("C1", nxt, n) for n in range(NCH))
            nxt += 1
        take = []
        while a_q and len(take) < 2:
            take.append(a_q.pop(0))
        room = (6 if len(c1_q) > 24 else 5) if (not a_q and len(c1_q) > 8) else (4 - len(take))
        while c1_q and len(take) < len(take) + room and room > 0:
            take.append(c1_q.pop(0))
            room -= 1
        plan[t]["pe"] = take
        for it in take:
            if it[0] == "C1" and it[2] == NCH - 1:
                done_step[it[1]] = t
    while nxt < NT:
        c1_q.extend(("C1", nxt, n) for n in range(NCH))
        nxt += 1
    tail["c1"] = [(j, n) for (_, j, n) in c1_q]
    att_t = {}
    for j in range(NT):
        d = done_step.get(j)
        t = None if d is None else d + 2
        while t is not None and t < T and plan[t]["attach"] is not None:
            t += 1
        if t is not None and t < T - 1:
            plan[t]["attach"] = j
            plan[t + 1]["c2"].extend((j, n) for n in range(C2N))
        else:
            tail["corr"].append(j)
            tail["c2"].extend((j, n) for n in range(C2N))
    return plan, tail


def _build(reps=1, do_c=True, use_cc=True):
    import concourse.bacc as bacc
    import concourse.tile as tile
    import concourse.mybir as mybir

    dt = mybir.dt
    AF = mybir.ActivationFunctionType
    ALU = mybir.AluOpType
    AX = mybir.AxisListType

    nc = bacc.Bacc("TRN2", target_bir_lowering=False, debug=False,
                   num_devices=NC)

    xT = nc.dram_tensor("xT", [H, B * T], dt.float16, kind="ExternalInput").ap()
    wihT = nc.dram_tensor("wihT", [H, 4 * HPC], dt.float16, kind="ExternalInput").ap()
    whhT = nc.dram_tensor("whhT", [H, 4 * HPC], dt.float16, kind="ExternalInput").ap()
    bias4 = nc.dram_tensor("bias4", [HPC, 4], dt.float32, kind="ExternalInput").ap()
    woutT = nc.dram_tensor("woutT", [H, VPC], dt.float16, kind="ExternalInput").ap()
    boutr = nc.dram_tensor("boutr", [1, VPC], dt.float16, kind="ExternalInput").ap()
    h0T = nc.dram_tensor("h0T", [H, B], dt.float16, kind="ExternalInput").ap()
    c0T = nc.dram_tensor("c0T", [HPC, B], dt.float32, kind="ExternalInput").ap()

    out_logp = nc.dram_tensor("out_logp", [B, T, VPC], dt.float32,
                              kind="ExternalOutput").ap()
    out_hT = nc.dram_tensor("out_hT", [HPC, B], dt.float32, kind="ExternalOutput").ap()
    out_cT = nc.dram_tensor("out_cT", [HPC, B], dt.float32, kind="ExternalOutput").ap()

    xg_dram = nc.dram_tensor("xg_dram", [128, 4, T, B], dt.float16).ap()

    plan, tail = _pacing()
    if not do_c:
        for t in plan:
            plan[t] = {"pe": [it for it in plan[t]["pe"] if it[0] == "A"],
                       "attach": None, "c2": []}
        tail = {"c1": [], "corr": [], "c2": []}
    rg = [list(range(NC))]

    with tile.TileContext(nc) as tc:
        with (
            tc.tile_pool(name="wsb", bufs=1) as wsb,
            tc.tile_pool(name="hst", bufs=NT) as hst,
            tc.tile_pool(name="p8k", bufs=5) as p8k,   # wih / xT chunks / logit ring
            tc.tile_pool(name="sta", bufs=2) as sta,
            tc.tile_pool(name="xtp", bufs=2) as xtp,
            tc.tile_pool(name="bwork", bufs=2) as bwork,
            tc.tile_pool(name="xgt", bufs=3) as xgt,
            tc.tile_pool(name="pay", bufs=2) as payp,
            tc.tile_pool(name="c1p", bufs=1) as c1p,
            tc.tile_pool(name="c2p", bufs=2) as c2p,
            tc.tile_pool(name="tiny", bufs=4) as tiny,
            tc.tile_pool(name="psA", bufs=2, space="PSUM") as psA,
            tc.tile_pool(name="psB", bufs=2, space="PSUM") as psB,
            tc.tile_pool(name="psC", bufs=2, space="PSUM") as psC,
            tc.tile_pool(name="dram", bufs=4, space="DRAM") as dram,
            tc.tile_pool(name="dramo", bufs=4, space="DRAM") as dramo,
        ):
            # ---- resident loads ----
            wih_sb = p8k.tile([128, KT, 4 * HPC], dt.float16, tag="p8k",
                              name="wih_sb")
            nc.sync.dma_start(wih_sb[:], wihT.rearrange("(k p) g -> p k g", p=128))
            whh_sb = wsb.tile([128, KT, 4 * HPC], dt.float16)
            nc.sync.dma_start(whh_sb[:], whhT.rearrange("(k p) g -> p k g", p=128))
            bias_sb = wsb.tile([HPC, 4], dt.float32)
            nc.sync.dma_start(bias_sb[:], bias4)
            wout_sb = wsb.tile([128, KT, VPC], dt.float16)
            bout_sb = wsb.tile([128, VPC], dt.float16)

            def load_wout(k):
                nc.sync.dma_start(
                    wout_sb[:, k, :],
                    woutT[k * 128:(k + 1) * 128, :])
            nc.sync.dma_start(bout_sb[:], boutr.broadcast_to([128, VPC]))
            h0_sb = wsb.tile([128, KT, B], dt.float16)
            nc.sync.dma_start(h0_sb[:], h0T.rearrange("(k p) b -> p k b", p=128))
            c_sb = wsb.tile([HPC, B], dt.float32)
            nc.sync.dma_start(c_sb[:], c0T)
            s8s = wsb.tile([128, NT, NCH], dt.float32)
            s_st = wsb.tile([128, NT], dt.float32)
            corr = wsb.tile([128, NT], dt.float32)

            for rep in range(reps):
                # ---- phase A helpers (256-col chunks, interleaved into B) ----
                xt_tiles = {}

                def emit_a(c, m):
                    c0 = c * 256
                    w = min(256, B * T - c0)
                    if m == 0:
                        xt_sb = xtp.tile([128, KT, 256], dt.float16, tag="xtp",
                                         name=f"xt{rep}_{c}")
                        nc.sync.dma_start(
                            xt_sb[:, :, :w],
                            xT[:, c0:c0 + w].rearrange("(k p) n -> p k n", p=128))
                        xt_tiles[c] = xt_sb
                    xt_sb = xt_tiles[c]
                    ps = psA.tile([128, 256], dt.float32, tag="psA",
                                  name=f"psa{rep}_{c}_{m}")
                    for k in range(KT):
                        nc.tensor.matmul(
                            ps[:, :w],
                            wih_sb[:, k, m * 128:(m + 1) * 128],
                            xt_sb[:, k, :w],
                            start=(k == 0), stop=(k == KT - 1))
                    st = sta.tile([128, 256], dt.float16, tag="sta",
                                  name=f"sta{rep}_{c}_{m}")
                    nc.vector.tensor_scalar_add(st[:, :w], ps[:, :w],
                                                bias_sb[:, m:m + 1])
                    nc.sync.dma_start(
                        xg_dram[:, m, c0 // B:(c0 + w) // B, :]
                        .rearrange("p t b -> p (t b)"),
                        st[:, :w])

                for c in (0, 1):
                    for m in range(4):
                        emit_a(c, m)

                hsT = [None] * NT
                ring = [None] * NT
                xg_tiles = {}

                def prefetch_xg(tp):
                    xg_sb = xgt.tile([128, 4, B], dt.float16, tag="xgt",
                                     name=f"xg{rep}_{tp}")
                    nc.sync.dma_start(xg_sb[:], xg_dram[:, :, tp, :])
                    xg_tiles[tp] = xg_sb

                def emit_c1(j, n):
                    ps = psC.tile([128, VCH], dt.float32, tag="psC",
                                  name=f"psc{rep}_{j}_{n}")
                    for k in range(KT):
                        nc.tensor.matmul(
                            ps[:],
                            hsT[j][:, k, :, :].rearrange("p t b -> p (t b)"),
                            wout_sb[:, k, n * VCH:(n + 1) * VCH],
                            start=(k == 0), stop=(k == KT - 1))
                    if n == 0:
                        ring[j] = p8k.tile([128, VPC], dt.float16, tag="p8k",
                                           name=f"ring{rep}_{j}")
                    sl = ring[j][:, n * VCH:(n + 1) * VCH]
                    nc.vector.tensor_add(sl, ps[:],
                                         bout_sb[:, n * VCH:(n + 1) * VCH])
                    scr = c1p.tile([128, VCH], dt.float16, tag="scr",
                                   name=f"scr{rep}_{j}_{n}")
                    nc.scalar.activation(scr[:], sl, AF.Exp,
                                         accum_out=s8s[:, j, n:n + 1])
                    if n == NCH - 1:
                        nc.vector.tensor_reduce(s_st[:, j:j + 1], s8s[:, j, :],
                                                axis=AX.X, op=ALU.add)

                def emit_corr(j, cc_out, col):
                    srec = tiny.tile([128, NC, 1], dt.float16, tag="srec",
                                     name=f"srec{rep}_{j}")
                    nc.sync.dma_start(
                        srec[:],
                        cc_out[:, col:col + 1].rearrange("(r p) c -> p r c", p=128))
                    stot = tiny.tile([128, 1], dt.float32, tag="stot",
                                     name=f"stot{rep}_{j}")
                    nc.vector.tensor_reduce(stot[:],
                                            srec[:].rearrange("p r c -> p (r c)"),
                                            axis=AX.X, op=ALU.add)
                    nc.scalar.activation(corr[:, j:j + 1], stot[:], AF.Ln)

                def emit_c2(j, n):
                    neg = tiny.tile([128, 1], dt.float32, tag="neg",
                                    name=f"neg{rep}_{j}_{n}")
                    nc.vector.tensor_scalar_mul(neg[:], corr[:, j:j + 1], -1.0)
                    so = c2p.tile([128, C2W], dt.float32, tag="c2o",
                                  name=f"c2o{rep}_{j}_{n}")
                    nc.vector.tensor_scalar_add(
                        so[:], ring[j][:, n * C2W:(n + 1) * C2W], neg[:])
                    nc.sync.dma_start(
                        out_logp.rearrange("b t v -> t b v")
                        [2 * j:2 * j + 2, :, n * C2W:(n + 1) * C2W],
                        so[:])

                # ---- phase B with interleaved A/C ----
                for t in range(T):
                    if t == 0:
                        for tp in (0, 1, 2):
                            prefetch_xg(tp)
                        for k in range(4):
                            load_wout(k)
                    elif t == 1:
                        for k in range(4, KT):
                            load_wout(k)
                    if t > 0 and t + 2 < T:
                        prefetch_xg(t + 2)
                    gp = psB.tile([128, 4 * B], dt.float32, tag="psB",
                                  name=f"gp{rep}_{t}")
                    for m in range(4):
                        for k in range(KT):
                            if t == 0:
                                rhs = h0_sb[:, k, :]
                            else:
                                pj, tau = divmod(t - 1, 2)
                                rhs = hsT[pj][:, k, tau, :]
                            nc.tensor.matmul(
                                gp[:, m * B:(m + 1) * B],
                                whh_sb[:, k, m * 128:(m + 1) * 128],
                                rhs, start=(k == 0), stop=(k == KT - 1))
                    gs = bwork.tile([128, 4 * B], dt.float32, tag="gs",
                                    name=f"gs{rep}_{t}")
                    nc.vector.tensor_add(gs[:], gp[:],
                                         xg_tiles[t][:].rearrange("p g b -> p (g b)"))
                    act = bwork.tile([128, 4 * B], dt.float32, tag="act",
                                     name=f"act{rep}_{t}")
                    nc.scalar.activation(act[:, 0:3 * B], gs[:, 0:3 * B], AF.Sigmoid)
                    nc.scalar.activation(act[:, 3 * B:4 * B], gs[:, 3 * B:4 * B],
                                         AF.Tanh)
                    ig = bwork.tile([128, B], dt.float32, tag="ig", name=f"ig{rep}_{t}")
                    nc.vector.tensor_mul(ig[:], act[:, 0:B], act[:, 3 * B:4 * B])
                    fc = bwork.tile([128, B], dt.float32, tag="fc", name=f"fc{rep}_{t}")
                    nc.vector.tensor_mul(fc[:], act[:, B:2 * B], c_sb[:])
                    nc.vector.tensor_add(c_sb[:], fc[:], ig[:])
                    tcc = bwork.tile([128, B], dt.float32, tag="tcc",
                                     name=f"tcc{rep}_{t}")
                    nc.scalar.activation(tcc[:], c_sb[:], AF.Tanh)

                    att = plan[t]["attach"]
                    w = 65 if att is not None else 64
                    pay = payp.tile([128, w], dt.float16, tag=f"pay{w}",
                                    name=f"pay{rep}_{t}")
                    nc.vector.tensor_mul(pay[:, 0:B], act[:, 2 * B:3 * B], tcc[:])

                    if t == T - 1:
                        h_t = bwork.tile([128, B], dt.float32, tag="ht",
                                         name=f"ht{rep}_{t}")
                        nc.vector.tensor_mul(h_t[:], act[:, 2 * B:3 * B], tcc[:])
                        nc.sync.dma_start(out_hT, h_t[:])
                        nc.sync.dma_start(out_cT, c_sb[:])
                    if att is not None:
                        nc.vector.tensor_copy(pay[:, 64:65], s_st[:, att:att + 1])
                    cc_in = dram.tile([128, w], dt.float16, tag=f"cin{w}",
                                      name=f"ccin{rep}_{t}")
                    nc.sync.dma_start(cc_in[:], pay[:])
                    cc_out = dramo.tile([NC * 128, w], dt.float16, tag=f"cout{w}",
                                        name=f"ccout{rep}_{t}", addr_space="Shared")
                    if use_cc:
                        nc.gpsimd.collective_compute(
                            "AllGather", mybir.AluOpType.bypass,
                            ins=[cc_in[:].opt()], outs=[cc_out[:].opt()],
                            replica_groups=rg)
                        recv_src = cc_out[:, 0:B].rearrange("(r p) b -> p r b", p=128)
                    else:
                        recv_src = cc_in[:, 0:B].rearrange(
                            "p (x b) -> p x b", x=1).broadcast_to([128, NC, B])
                    j, tau = divmod(t, 2)
                    if tau == 0:
                        hsT[j] = hst.tile([128, KT, 2, B], dt.float16, tag="hsT",
                                          name=f"hsT{rep}_{j}")
                    nc.sync.dma_start(hsT[j][:, :, tau, :], recv_src)
                    if att is not None:
                        if use_cc:
                            emit_corr(att, cc_out, 64)
                        else:
                            nc.vector.tensor_copy(corr[:, att:att + 1],
                                                  s_st[:, att:att + 1])
                    for it in plan[t]["pe"]:
                        if it[0] == "A":
                            emit_a(it[1], it[2])
                        else:
                            emit_c1(it[1], it[2])
                    for (cj, cn) in plan[t]["c2"]:
                        emit_c2(cj, cn)

                # ---- tail ----
                for (cj, cn) in tail["c1"]:
                    emit_c1(cj, cn)
                ntail = len(tail["corr"])
                if ntail and not use_cc:
                    for j in tail["corr"]:
                        nc.vector.tensor_copy(corr[:, j:j + 1], s_st[:, j:j + 1])
                if ntail and use_cc:
                    payt = payp.tile([128, ntail], dt.float16, tag="payt",
                                     name=f"payt{rep}")
                    for i, j in enumerate(tail["corr"]):
                        nc.vector.tensor_copy(payt[:, i:i + 1], s_st[:, j:j + 1])
                    cc_in = dram.tile([128, ntail], dt.float16, tag="cint",
                                      name=f"ccint{rep}")
                    nc.sync.dma_start(cc_in[:], payt[:])
                    cc_out = dramo.tile([NC * 128, ntail], dt.float16, tag="coutt",
                                        name=f"ccoutt{rep}", addr_space="Shared")
                    nc.gpsimd.collective_compute(
                        "AllGather", mybir.AluOpType.bypass,
                        ins=[cc_in[:].opt()], outs=[cc_out[:].opt()],
                        replica_groups=rg)
                    for i, j in enumerate(tail["corr"]):
                        emit_corr(j, cc_out, i)
                for (cj, cn) in tail["c2"]:
                    emit_c2(cj, cn)

    nc.compile()
    return nc


def _prep(inputs):
    f16 = np.float16
    emb = inputs["emb"]
    tgt = inputs["target_tensor"]
    tokens = np.concatenate(
        [np.full((B, 1), SOS, tgt.dtype), tgt[:, :T - 1]], axis=1)
    x = np.maximum(emb[tokens], 0.0)
    xT16 = np.ascontiguousarray(x.transpose(2, 1, 0).reshape(H, T * B)).astype(f16)
    bsum = (inputs["b_ih"] + inputs["b_hh"]).astype(np.float32)
    W_ih, W_hh = inputs["W_ih"], inputs["W_hh"]
    W_out, b_out = inputs["W_out"], inputs["b_out"]
    h0 = inputs["encoder_hidden_h"][0]
    c0 = inputs["encoder_hidden_c"][0]
    h0T16 = np.ascontiguousarray(h0.T).astype(f16)
    c0T = np.ascontiguousarray(c0.T).astype(np.float32)

    in_maps = []
    for k in range(NC):
        rows = np.concatenate([o * H + np.arange(k * HPC, (k + 1) * HPC)
                               for o in (0, 1, 3, 2)])
        in_maps.append({
            "xT": xT16,
            "wihT": np.ascontiguousarray(W_ih[rows].T).astype(f16),
            "whhT": np.ascontiguousarray(W_hh[rows].T).astype(f16),
            "bias4": np.ascontiguousarray(bsum[rows].reshape(4, HPC).T),
            "woutT": np.ascontiguousarray(W_out[k * VPC:(k + 1) * VPC].T).astype(f16),
            "boutr": b_out[k * VPC:(k + 1) * VPC].reshape(1, VPC).astype(f16),
            "h0T": h0T16,
            "c0T": np.ascontiguousarray(c0T[k * HPC:(k + 1) * HPC]),
        })
    return in_maps


def kernel(**inputs):
    global _compiled
    from concourse.bass_utils import run_bass_kernel_spmd
    inputs = {k: np.asarray(v) for k, v in inputs.items()}
    if _compiled is None:
        _compiled = _build()
    in_maps = _prep(inputs)
    res = run_bass_kernel_spmd(_compiled, in_maps, list(range(NC)))
    r = res.results
    logp = np.concatenate([r[k]["out_logp"] for k in range(NC)], axis=2)
    hT = np.zeros((1, B, H), np.float32)
    cT = np.zeros((1, B, H), np.float32)
    for k in range(NC):
        hT[0, :, k * HPC:(k + 1) * HPC] = r[k]["out_hT"].T
        cT[0, :, k * HPC:(k + 1) * HPC] = r[k]["out_cT"].T
    return logp, hT, cT


# revision 12
# speedup vs baseline: 1.0125x; 1.0125x over previous
"""DecoderRNN (LSTM + vocab projection + log_softmax) on 8 Trainium2 cores.

Strategy (self-contained; B=64, T=50, H=1024, V=32000 hardcoded):
  - Host: token shift, embedding gather + relu, transposes/casts to fp16,
    per-core weight slicing.
  - Phase A: x_gates = W_ih_k @ x^T + b_k (gate-dim sharded), staged to DRAM.
  - Phase B: LSTM recurrence, tensor-parallel over the hidden dim; core k
    owns h/c slice [128]. Per step: gates^T = W_hh_k @ h^T (fp16 operands,
    f32 psum), nonlinearities, AllGather of h_k^T across 8 cores (50 AGs).
  - Phase C (interleaved into B's AllGather latency): logits for the core's
    4000-vocab slice in [128bt, 4000] tiles held fp16 in an SBUF ring;
    sum-exp via ACT accum_out (logits are O(1): no max subtraction needed);
    per-tile sums piggybacked on the phase-B AllGathers; logp written
    directly from the ring once the global sum arrives.
  - Host: concat vocab slices; assemble hT/cT from per-core slices.
"""
import numpy as np

B, T, H, V = 64, 50, 1024, 32000
NC = 8
HPC = H // NC          # 128
VPC = V // NC          # 4000
KT = H // 128          # 8
NT = (B * T) // 128    # 25 bt tiles (t-major: col = t*64 + b)
NCH = 8                # C1 psum chunks per bt tile (500 wide)
VCH = VPC // NCH       # 500
C2N = 4                # C2 chunks per tile
C2W = VPC // C2N       # 1000
SOS = 0

_compiled = None


def _pacing():
    """Unified schedule. Returns per-step dict with:
       pe: list of ("A", chunk, m) | ("C1", j, n) work items
       attach: tile j whose sumexp rides this step's AllGather (or None)
       c2: list of (j, n) logp-write chunks
    plus upfront A pairs and tail lists. A chunks are 256 bt-cols wide.
    """
    NAC = 13                     # A chunks: 12*256 + 1*128
    a_q = [("A", c, m) for c in range(2, NAC) for m in range(4)]
    plan = {t: {"pe": [], "attach": None, "c2": []} for t in range(T)}
    tail = {"c1": [], "corr": [], "c2": []}
    c1_q = []
    nxt = 0
    done_step = {}
    for t in range(T):
        while nxt < NT and t >= 2 * nxt + 3:
            c1_q.extend(("C1", nxt, n) for n in range(NCH))
            nxt += 1
        take = []
        while a_q and len(take) < 2:
            take.append(a_q.pop(0))
        room = 0 if t < 5 else (5 if (not a_q and len(c1_q) > 20) else 4 - len(take))
        while c1_q and room > 0:
            take.append(c1_q.pop(0))
            room -= 1
        plan[t]["pe"] = take
        for it in take:
            if it[0] == "C1" and it[2] == NCH - 1:
                done_step[it[1]] = t
    while nxt < NT:
        c1_q.extend(("C1", nxt, n) for n in range(NCH))
        nxt += 1
    tail["c1"] = [(j, n) for (_, j, n) in c1_q]
    att_t = {}
    for j in range(NT):
        d = done_step.get(j)
        t = None if d is None else d + 2
        while t is not None and t < T and plan[t]["attach"] is not None:
            t += 1
        if t is not None and t < T - 1:
            plan[t]["attach"] = j
            plan[t + 1]["c2"].extend((j, n) for n in range(C2N))
        else:
            tail["corr"].append(j)
            tail["c2"].extend((j, n) for n in range(C2N))
    return plan, tail


def _build(reps=1, do_c=True, use_cc=True):
    import concourse.bacc as bacc
    import concourse.tile as tile
    import concourse.mybir as mybir

    dt = mybir.dt
    AF = mybir.ActivationFunctionType
    ALU = mybir.AluOpType
    AX = mybir.AxisListType

    nc = bacc.Bacc("TRN2", target_bir_lowering=False, debug=False,
                   num_devices=NC)

    xT = nc.dram_tensor("xT", [H, B * T], dt.float16, kind="ExternalInput").ap()
    wihT = nc.dram_tensor("wihT", [H, 4 * HPC], dt.float16, kind="ExternalInput").ap()
    whhT = nc.dram_tensor("whhT", [H, 4 * HPC], dt.float16, kind="ExternalInput").ap()
    bias4 = nc.dram_tensor("bias4", [HPC, 4], dt.float32, kind="ExternalInput").ap()
    woutT = nc.dram_tensor("woutT", [H, VPC], dt.float16, kind="ExternalInput").ap()
    boutr = nc.dram_tensor("boutr", [1, VPC], dt.float16, kind="ExternalInput").ap()
    h0T = nc.dram_tensor("h0T", [H, B], dt.float16, kind="ExternalInput").ap()
    c0T = nc.dram_tensor("c0T", [HPC, B], dt.float32, kind="ExternalInput").ap()

    out_logp = nc.dram_tensor("out_logp", [B, T, VPC], dt.float32,
                              kind="ExternalOutput").ap()
    out_hT = nc.dram_tensor("out_hT", [HPC, B], dt.float32, kind="ExternalOutput").ap()
    out_cT = nc.dram_tensor("out_cT", [HPC, B], dt.float32, kind="ExternalOutput").ap()

    xg_dram = nc.dram_tensor("xg_dram", [128, 4, T, B], dt.float16).ap()

    plan, tail = _pacing()
    if not do_c:
        for t in plan:
            plan[t] = {"pe": [it for it in plan[t]["pe"] if it[0] == "A"],
                       "attach": None, "c2": []}
        tail = {"c1": [], "corr": [], "c2": []}
    rg = [list(range(NC))]

    with tile.TileContext(nc) as tc:
        with (
            tc.tile_pool(name="wsb", bufs=1) as wsb,
            tc.tile_pool(name="hst", bufs=NT) as hst,
            tc.tile_pool(name="p8k", bufs=5) as p8k,   # wih / xT chunks / logit ring
            tc.tile_pool(name="sta", bufs=2) as sta,
            tc.tile_pool(name="xtp", bufs=2) as xtp,
            tc.tile_pool(name="bwork", bufs=2) as bwork,
            tc.tile_pool(name="xgt", bufs=3) as xgt,
            tc.tile_pool(name="pay", bufs=2) as payp,
            tc.tile_pool(name="c1p", bufs=1) as c1p,
            tc.tile_pool(name="c2p", bufs=2) as c2p,
            tc.tile_pool(name="tiny", bufs=4) as tiny,
            tc.tile_pool(name="psA", bufs=2, space="PSUM") as psA,
            tc.tile_pool(name="psB", bufs=2, space="PSUM") as psB,
            tc.tile_pool(name="psC", bufs=2, space="PSUM") as psC,
            tc.tile_pool(name="dram", bufs=4, space="DRAM") as dram,
            tc.tile_pool(name="dramo", bufs=4, space="DRAM") as dramo,
        ):
            # ---- resident loads ----
            whh_sb = wsb.tile([128, KT, 4 * HPC], dt.float16)
            nc.sync.dma_start(whh_sb[:], whhT.rearrange("(k p) g -> p k g", p=128))
            bias_sb = wsb.tile([HPC, 4], dt.float32)
            nc.sync.dma_start(bias_sb[:], bias4)
            wout_sb = wsb.tile([128, KT, VPC], dt.float16)
            bout_sb = wsb.tile([128, VPC], dt.float16)

            def load_wout(k):
                nc.sync.dma_start(
                    wout_sb[:, k, :],
                    woutT[k * 128:(k + 1) * 128, :])
            nc.sync.dma_start(bout_sb[:], boutr.broadcast_to([128, VPC]))
            h0_sb = wsb.tile([128, KT, B], dt.float16)
            nc.sync.dma_start(h0_sb[:], h0T.rearrange("(k p) b -> p k b", p=128))
            c_sb = wsb.tile([HPC, B], dt.float32)
            nc.sync.dma_start(c_sb[:], c0T)
            s8s = wsb.tile([128, NT, NCH], dt.float32)
            s_st = wsb.tile([128, NT], dt.float32)
            corr = wsb.tile([128, NT], dt.float32)

            for rep in range(reps):
                # ---- phase A helpers (256-col chunks, interleaved into B) ----
                wih_sb = p8k.tile([128, KT, 4 * HPC], dt.float16, tag="p8k",
                                  name=f"wih_sb{rep}")
                nc.sync.dma_start(wih_sb[:],
                                  wihT.rearrange("(k p) g -> p k g", p=128))
                xt_tiles = {}

                def emit_a(c, m):
                    c0 = c * 256
                    w = min(256, B * T - c0)
                    if m == 0:
                        xt_sb = xtp.tile([128, KT, 256], dt.float16, tag="xtp",
                                         name=f"xt{rep}_{c}")
                        nc.sync.dma_start(
                            xt_sb[:, :, :w],
                            xT[:, c0:c0 + w].rearrange("(k p) n -> p k n", p=128))
                        xt_tiles[c] = xt_sb
                    xt_sb = xt_tiles[c]
                    ps = psA.tile([128, 256], dt.float32, tag="psA",
                                  name=f"psa{rep}_{c}_{m}")
                    for k in range(KT):
                        nc.tensor.matmul(
                            ps[:, :w],
                            wih_sb[:, k, m * 128:(m + 1) * 128],
                            xt_sb[:, k, :w],
                            start=(k == 0), stop=(k == KT - 1))
                    st = sta.tile([128, 256], dt.float16, tag="sta",
                                  name=f"sta{rep}_{c}_{m}")
                    nc.vector.tensor_scalar_add(st[:, :w], ps[:, :w],
                                                bias_sb[:, m:m + 1])
                    nc.sync.dma_start(
                        xg_dram[:, m, c0 // B:(c0 + w) // B, :]
                        .rearrange("p t b -> p (t b)"),
                        st[:, :w])

                for c in (0, 1):
                    for m in range(4):
                        emit_a(c, m)

                hsT = [None] * NT
                ring = [None] * NT
                xg_tiles = {}

                def prefetch_xg(tp):
                    xg_sb = xgt.tile([128, 4, B], dt.float16, tag="xgt",
                                     name=f"xg{rep}_{tp}")
                    nc.sync.dma_start(xg_sb[:], xg_dram[:, :, tp, :])
                    xg_tiles[tp] = xg_sb

                def emit_c1(j, n):
                    ps = psC.tile([128, VCH], dt.float32, tag="psC",
                                  name=f"psc{rep}_{j}_{n}")
                    for k in range(KT):
                        nc.tensor.matmul(
                            ps[:],
                            hsT[j][:, k, :, :].rearrange("p t b -> p (t b)"),
                            wout_sb[:, k, n * VCH:(n + 1) * VCH],
                            start=(k == 0), stop=(k == KT - 1))
                    if n == 0:
                        ring[j] = p8k.tile([128, VPC], dt.float16, tag="p8k",
                                           name=f"ring{rep}_{j}")
                    sl = ring[j][:, n * VCH:(n + 1) * VCH]
                    nc.vector.tensor_add(sl, ps[:],
                                         bout_sb[:, n * VCH:(n + 1) * VCH])
                    if n == NCH - 1:
                        scr = c1p.tile([128, VPC], dt.float16, tag="scr",
                                       name=f"scr{rep}_{j}")
                        nc.scalar.activation(scr[:], ring[j][:], AF.Exp,
                                             accum_out=s_st[:, j:j + 1])

                def emit_corr(j, cc_out, col):
                    srec = tiny.tile([128, NC, 1], dt.float16, tag="srec",
                                     name=f"srec{rep}_{j}")
                    nc.sync.dma_start(
                        srec[:],
                        cc_out[:, col:col + 1].rearrange("(r p) c -> p r c", p=128))
                    stot = tiny.tile([128, 1], dt.float32, tag="stot",
                                     name=f"stot{rep}_{j}")
                    nc.vector.tensor_reduce(stot[:],
                                            srec[:].rearrange("p r c -> p (r c)"),
                                            axis=AX.X, op=ALU.add)
                    nc.scalar.activation(corr[:, j:j + 1], stot[:], AF.Ln)

                def emit_c2(j, n):
                    neg = tiny.tile([128, 1], dt.float32, tag="neg",
                                    name=f"neg{rep}_{j}_{n}")
                    nc.vector.tensor_scalar_mul(neg[:], corr[:, j:j + 1], -1.0)
                    so = c2p.tile([128, C2W], dt.float32, tag="c2o",
                                  name=f"c2o{rep}_{j}_{n}")
                    nc.vector.tensor_scalar_add(
                        so[:], ring[j][:, n * C2W:(n + 1) * C2W], neg[:])
                    nc.sync.dma_start(
                        out_logp.rearrange("b t v -> t b v")
                        [2 * j:2 * j + 2, :, n * C2W:(n + 1) * C2W],
                        so[:])

                # ---- phase B with interleaved A/C ----
                for t in range(T):
                    if t == 0:
                        for tp in (0, 1, 2):
                            prefetch_xg(tp)
                        for k in range(4):
                            load_wout(k)
                    elif t == 1:
                        for k in range(4, KT):
                            load_wout(k)
                    if t > 0 and t + 2 < T:
                        prefetch_xg(t + 2)
                    gp = psB.tile([128, 4 * B], dt.float32, tag="psB",
                                  name=f"gp{rep}_{t}")
                    for m in range(4):
                        for k in range(KT):
                            if t == 0:
                                rhs = h0_sb[:, k, :]
                            else:
                                pj, tau = divmod(t - 1, 2)
                                rhs = hsT[pj][:, k, tau, :]
                            nc.tensor.matmul(
                                gp[:, m * B:(m + 1) * B],
                                whh_sb[:, k, m * 128:(m + 1) * 128],
                                rhs, start=(k == 0), stop=(k == KT - 1))
                    gs = bwork.tile([128, 4 * B], dt.float32, tag="gs",
                                    name=f"gs{rep}_{t}")
                    nc.vector.tensor_add(gs[:], gp[:],
                                         xg_tiles[t][:].rearrange("p g b -> p (g b)"))
                    act = bwork.tile([128, 4 * B], dt.float32, tag="act",
                                     name=f"act{rep}_{t}")
                    nc.scalar.activation(act[:, 0:3 * B], gs[:, 0:3 * B], AF.Sigmoid)
                    nc.scalar.activation(act[:, 3 * B:4 * B], gs[:, 3 * B:4 * B],
                                         AF.Sigmoid, scale=2.0)
                    gt = bwork.tile([128, B], dt.float32, tag="gt", name=f"gt{rep}_{t}")
                    nc.vector.tensor_scalar(gt[:], act[:, 3 * B:4 * B], 2.0, -1.0,
                                            op0=ALU.mult, op1=ALU.add)
                    ig = bwork.tile([128, B], dt.float32, tag="ig", name=f"ig{rep}_{t}")
                    nc.vector.tensor_mul(ig[:], act[:, 0:B], gt[:])
                    fc = bwork.tile([128, B], dt.float32, tag="fc", name=f"fc{rep}_{t}")
                    nc.vector.tensor_mul(fc[:], act[:, B:2 * B], c_sb[:])
                    nc.vector.tensor_add(c_sb[:], fc[:], ig[:])
                    tcs = bwork.tile([128, B], dt.float32, tag="tcs",
                                     name=f"tcs{rep}_{t}")
                    nc.scalar.activation(tcs[:], c_sb[:], AF.Sigmoid, scale=2.0)
                    tcc = bwork.tile([128, B], dt.float32, tag="tcc",
                                     name=f"tcc{rep}_{t}")
                    nc.vector.tensor_scalar(tcc[:], tcs[:], 2.0, -1.0,
                                            op0=ALU.mult, op1=ALU.add)

                    att = plan[t]["attach"]
                    w = 65 if att is not None else 64
                    pay = payp.tile([128, w], dt.float16, tag=f"pay{w}",
                                    name=f"pay{rep}_{t}")
                    nc.vector.tensor_mul(pay[:, 0:B], act[:, 2 * B:3 * B], tcc[:])

                    if t == T - 1:
                        h_t = bwork.tile([128, B], dt.float32, tag="ht",
                                         name=f"ht{rep}_{t}")
                        nc.vector.tensor_mul(h_t[:], act[:, 2 * B:3 * B], tcc[:])
                        nc.sync.dma_start(out_hT, h_t[:])
                        nc.sync.dma_start(out_cT, c_sb[:])
                    if att is not None:
                        nc.vector.tensor_copy(pay[:, 64:65], s_st[:, att:att + 1])
                    cc_in = dram.tile([128, w], dt.float16, tag=f"cin{w}",
                                      name=f"ccin{rep}_{t}")
                    nc.sync.dma_start(cc_in[:], pay[:])
                    cc_out = dramo.tile([NC * 128, w], dt.float16, tag=f"cout{w}",
                                        name=f"ccout{rep}_{t}", addr_space="Shared")
                    if use_cc:
                        nc.gpsimd.collective_compute(
                            "AllGather", mybir.AluOpType.bypass,
                            ins=[cc_in[:].opt()], outs=[cc_out[:].opt()],
                            replica_groups=rg)
                        recv_src = cc_out[:, 0:B].rearrange("(r p) b -> p r b", p=128)
                    else:
                        recv_src = cc_in[:, 0:B].rearrange(
                            "p (x b) -> p x b", x=1).broadcast_to([128, NC, B])
                    j, tau = divmod(t, 2)
                    if tau == 0:
                        hsT[j] = hst.tile([128, KT, 2, B], dt.float16, tag="hsT",
                                          name=f"hsT{rep}_{j}")
                    nc.sync.dma_start(hsT[j][:, :, tau, :], recv_src)
                    for it in plan[t]["pe"]:
                        if it[0] == "A":
                            emit_a(it[1], it[2])
                        else:
                            emit_c1(it[1], it[2])
                    if att is not None:
                        if use_cc:
                            emit_corr(att, cc_out, 64)
                        else:
                            nc.vector.tensor_copy(corr[:, att:att + 1],
                                                  s_st[:, att:att + 1])
                    for (cj, cn) in plan[t]["c2"]:
                        emit_c2(cj, cn)

                # ---- tail ----
                for (cj, cn) in tail["c1"]:
                    emit_c1(cj, cn)
                ntail = len(tail["corr"])
                if ntail and not use_cc:
                    for j in tail["corr"]:
                        nc.vector.tensor_copy(corr[:, j:j + 1], s_st[:, j:j + 1])
                if ntail and use_cc:
                    payt = payp.tile([128, ntail], dt.float16, tag="payt",
                                     name=f"payt{rep}")
                    for i, j in enumerate(tail["corr"]):
                        nc.vector.tensor_copy(payt[:, i:i + 1], s_st[:, j:j + 1])
                    cc_in = dram.tile([128, ntail], dt.float16, tag="cint",
                                      name=f"ccint{rep}")
                    nc.sync.dma_start(cc_in[:], payt[:])
                    cc_out = dramo.tile([NC * 128, ntail], dt.float16, tag="coutt",
                                        name=f"ccoutt{rep}", addr_space="Shared")
                    nc.gpsimd.collective_compute(
                        "AllGather", mybir.AluOpType.bypass,
                        ins=[cc_in[:].opt()], outs=[cc_out[:].opt()],
                        replica_groups=rg)
                    for i, j in enumerate(tail["corr"]):
                        emit_corr(j, cc_out, i)
                for (cj, cn) in tail["c2"]:
                    emit_c2(cj, cn)

    nc.compile()
    return nc


def _prep(inputs):
    f16 = np.float16
    emb = inputs["emb"]
    tgt = inputs["target_tensor"]
    tokens = np.concatenate(
        [np.full((B, 1), SOS, tgt.dtype), tgt[:, :T - 1]], axis=1)
    x = np.maximum(emb[tokens], 0.0)
    xT16 = np.ascontiguousarray(x.transpose(2, 1, 0).reshape(H, T * B)).astype(f16)
    bsum = (inputs["b_ih"] + inputs["b_hh"]).astype(np.float32)
    W_ih, W_hh = inputs["W_ih"], inputs["W_hh"]
    W_out, b_out = inputs["W_out"], inputs["b_out"]
    h0 = inputs["encoder_hidden_h"][0]
    c0 = inputs["encoder_hidden_c"][0]
    h0T16 = np.ascontiguousarray(h0.T).astype(f16)
    c0T = np.ascontiguousarray(c0.T).astype(np.float32)

    in_maps = []
    for k in range(NC):
        rows = np.concatenate([o * H + np.arange(k * HPC, (k + 1) * HPC)
                               for o in (0, 1, 3, 2)])
        in_maps.append({
            "xT": xT16,
            "wihT": np.ascontiguousarray(W_ih[rows].T).astype(f16),
            "whhT": np.ascontiguousarray(W_hh[rows].T).astype(f16),
            "bias4": np.ascontiguousarray(bsum[rows].reshape(4, HPC).T),
            "woutT": np.ascontiguousarray(W_out[k * VPC:(k + 1) * VPC].T).astype(f16),
            "boutr": b_out[k * VPC:(k + 1) * VPC].reshape(1, VPC).astype(f16),
            "h0T": h0T16,
            "c0T": np.ascontiguousarray(c0T[k * HPC:(k + 1) * HPC]),
        })
    return in_maps


def kernel(**inputs):
    global _compiled
    from concourse.bass_utils import run_bass_kernel_spmd
    inputs = {k: np.asarray(v) for k, v in inputs.items()}
    if _compiled is None:
        _compiled = _build()
    in_maps = _prep(inputs)
    res = run_bass_kernel_spmd(_compiled, in_maps, list(range(NC)))
    r = res.results
    logp = np.concatenate([r[k]["out_logp"] for k in range(NC)], axis=2)
    hT = np.zeros((1, B, H), np.float32)
    cT = np.zeros((1, B, H), np.float32)
    for k in range(NC):
        hT[0, :, k * HPC:(k + 1) * HPC] = r[k]["out_hT"].T
        cT[0, :, k * HPC:(k + 1) * HPC] = r[k]["out_cT"].T
    return logp, hT, cT


# revision 13
# speedup vs baseline: 1.1507x; 1.1365x over previous
"""DecoderRNN (LSTM + vocab projection + log_softmax) on 8 Trainium2 cores.

Strategy (self-contained; B=64, T=50, H=1024, V=32000 hardcoded):
  - Host: token shift, embedding gather + relu, transposes/casts to fp16,
    per-core weight slicing.
  - Phase A: x_gates = W_ih_k @ x^T + b_k (gate-dim sharded), staged to DRAM.
  - Phase B: LSTM recurrence, tensor-parallel over the hidden dim; core k
    owns h/c slice [128]. Per step: gates^T = W_hh_k @ h^T (fp16 operands,
    f32 psum), nonlinearities, AllGather of h_k^T across 8 cores (50 AGs).
  - Phase C (interleaved into B's AllGather latency): logits for the core's
    4000-vocab slice in [128bt, 4000] tiles held fp16 in an SBUF ring;
    sum-exp via ACT accum_out (logits are O(1): no max subtraction needed);
    per-tile sums piggybacked on the phase-B AllGathers; logp written
    directly from the ring once the global sum arrives.
  - Host: concat vocab slices; assemble hT/cT from per-core slices.
"""
import numpy as np

B, T, H, V = 64, 50, 1024, 32000
NC = 8
HPC = H // NC          # 128
VPC = V // NC          # 4000
KT = H // 128          # 8
NT = (B * T) // 128    # 25 bt tiles (t-major: col = t*64 + b)
NCH = 8                # C1 psum chunks per bt tile (500 wide)
VCH = VPC // NCH       # 500
C2N = 4                # C2 chunks per tile
C2W = VPC // C2N       # 1000
SOS = 0

_compiled = None


def _pacing():
    """Unified schedule. Returns per-step dict with:
       pe: list of ("A", chunk, m) | ("C1", j, n) work items
       attach: tile j whose sumexp rides this step's AllGather (or None)
       c2: list of (j, n) logp-write chunks
    plus upfront A pairs and tail lists. A chunks are 256 bt-cols wide.
    """
    NAC = 13                     # A chunks: 12*256 + 1*128
    a_q = [("A", c, m) for c in range(2, NAC) for m in range(4)]
    plan = {t: {"pe": [], "attach": None, "c2": []} for t in range(T)}
    tail = {"c1": [], "corr": [], "c2": []}
    c1_q = []
    nxt = 0
    done_step = {}
    for t in range(T):
        while nxt < NT and t >= 2 * nxt + 3:
            c1_q.extend(("C1", nxt, n) for n in range(NCH))
            nxt += 1
        take = []
        while a_q and len(take) < 2:
            take.append(a_q.pop(0))
        room = 0 if t < 5 else (5 if (not a_q and len(c1_q) > 20) else 4 - len(take))
        while c1_q and room > 0:
            take.append(c1_q.pop(0))
            room -= 1
        plan[t]["pe"] = take
        for it in take:
            if it[0] == "C1" and it[2] == NCH - 1:
                done_step[it[1]] = t
    while nxt < NT:
        c1_q.extend(("C1", nxt, n) for n in range(NCH))
        nxt += 1
    tail["c1"] = [(j, n) for (_, j, n) in c1_q]
    att_t = {}
    for j in range(NT):
        d = done_step.get(j)
        t = None if d is None else d + 2
        while t is not None and t < T and plan[t]["attach"] is not None:
            t += 1
        if t is not None and t < T - 1:
            plan[t]["attach"] = j
            plan[t + 1]["c2"].extend((j, n) for n in range(C2N))
        else:
            tail["corr"].append(j)
            tail["c2"].extend((j, n) for n in range(C2N))
    return plan, tail


def _build(reps=1, do_c=True, use_cc=True):
    import concourse.bacc as bacc
    import concourse.tile as tile
    import concourse.mybir as mybir

    dt = mybir.dt
    AF = mybir.ActivationFunctionType
    ALU = mybir.AluOpType
    AX = mybir.AxisListType

    nc = bacc.Bacc("TRN2", target_bir_lowering=False, debug=False,
                   num_devices=NC)

    xT = nc.dram_tensor("xT", [H, B * T], dt.float16, kind="ExternalInput").ap()
    wihT = nc.dram_tensor("wihT", [H, 4 * HPC], dt.float16, kind="ExternalInput").ap()
    whhT = nc.dram_tensor("whhT", [H, 4 * HPC], dt.float16, kind="ExternalInput").ap()
    bias4 = nc.dram_tensor("bias4", [HPC, 4], dt.float32, kind="ExternalInput").ap()
    woutT = nc.dram_tensor("woutT", [H, VPC], dt.float16, kind="ExternalInput").ap()
    boutr = nc.dram_tensor("boutr", [1, VPC], dt.float16, kind="ExternalInput").ap()
    h0T = nc.dram_tensor("h0T", [H, B], dt.float16, kind="ExternalInput").ap()
    c0T = nc.dram_tensor("c0T", [HPC, B], dt.float32, kind="ExternalInput").ap()

    out_logp = nc.dram_tensor("out_logp", [B, T, VPC], dt.float32,
                              kind="ExternalOutput").ap()
    out_hT = nc.dram_tensor("out_hT", [HPC, B], dt.float32, kind="ExternalOutput").ap()
    out_cT = nc.dram_tensor("out_cT", [HPC, B], dt.float32, kind="ExternalOutput").ap()

    xg_dram = nc.dram_tensor("xg_dram", [128, 4, T, B], dt.float16).ap()

    plan, tail = _pacing()
    if not do_c:
        for t in plan:
            plan[t] = {"pe": [it for it in plan[t]["pe"] if it[0] == "A"],
                       "attach": None, "c2": []}
        tail = {"c1": [], "corr": [], "c2": []}
    rg = [list(range(NC))]

    with tile.TileContext(nc) as tc:
        with (
            tc.tile_pool(name="wsb", bufs=1) as wsb,
            tc.tile_pool(name="hst", bufs=NT) as hst,
            tc.tile_pool(name="p8k", bufs=5) as p8k,   # wih / xT chunks / logit ring
            tc.tile_pool(name="sta", bufs=2) as sta,
            tc.tile_pool(name="xtp", bufs=2) as xtp,
            tc.tile_pool(name="bwork", bufs=2) as bwork,
            tc.tile_pool(name="xgt", bufs=3) as xgt,
            tc.tile_pool(name="pay", bufs=2) as payp,
            tc.tile_pool(name="c1p", bufs=1) as c1p,
            tc.tile_pool(name="c2p", bufs=2) as c2p,
            tc.tile_pool(name="tiny", bufs=4) as tiny,
            tc.tile_pool(name="psA", bufs=2, space="PSUM") as psA,
            tc.tile_pool(name="psB", bufs=2, space="PSUM") as psB,
            tc.tile_pool(name="psC", bufs=4, space="PSUM") as psC,
            tc.tile_pool(name="dram", bufs=4, space="DRAM") as dram,
            tc.tile_pool(name="dramo", bufs=4, space="DRAM") as dramo,
        ):
            # ---- resident loads ----
            whh_sb = wsb.tile([128, KT, 4 * HPC], dt.float16)
            nc.sync.dma_start(whh_sb[:], whhT.rearrange("(k p) g -> p k g", p=128))
            bias_sb = wsb.tile([HPC, 4], dt.float32)
            nc.sync.dma_start(bias_sb[:], bias4)
            wout_sb = wsb.tile([128, KT, VPC], dt.float16)
            bout_sb = wsb.tile([128, VPC], dt.float16)

            def load_wout(k):
                nc.sync.dma_start(
                    wout_sb[:, k, :],
                    woutT[k * 128:(k + 1) * 128, :])
            nc.sync.dma_start(bout_sb[:], boutr.broadcast_to([128, VPC]))
            h0_sb = wsb.tile([128, KT, B], dt.float16)
            nc.sync.dma_start(h0_sb[:], h0T.rearrange("(k p) b -> p k b", p=128))
            c_sb = wsb.tile([HPC, B], dt.float32)
            nc.sync.dma_start(c_sb[:], c0T)
            s8s = wsb.tile([128, NT, NCH], dt.float32)
            s_st = wsb.tile([128, NT], dt.float32)
            corr = wsb.tile([128, NT], dt.float32)

            for rep in range(reps):
                # ---- phase A helpers (256-col chunks, interleaved into B) ----
                wih_sb = p8k.tile([128, KT, 4 * HPC], dt.float16, tag="p8k",
                                  name=f"wih_sb{rep}")
                nc.sync.dma_start(wih_sb[:],
                                  wihT.rearrange("(k p) g -> p k g", p=128))
                xt_tiles = {}

                def emit_a(c, m):
                    c0 = c * 256
                    w = min(256, B * T - c0)
                    if m == 0:
                        xt_sb = xtp.tile([128, KT, 256], dt.float16, tag="xtp",
                                         name=f"xt{rep}_{c}")
                        nc.sync.dma_start(
                            xt_sb[:, :, :w],
                            xT[:, c0:c0 + w].rearrange("(k p) n -> p k n", p=128))
                        xt_tiles[c] = xt_sb
                    xt_sb = xt_tiles[c]
                    ps = psA.tile([128, 256], dt.float32, tag="psA",
                                  name=f"psa{rep}_{c}_{m}")
                    for k in range(KT):
                        nc.tensor.matmul(
                            ps[:, :w],
                            wih_sb[:, k, m * 128:(m + 1) * 128],
                            xt_sb[:, k, :w],
                            start=(k == 0), stop=(k == KT - 1))
                    st = sta.tile([128, 256], dt.float16, tag="sta",
                                  name=f"sta{rep}_{c}_{m}")
                    nc.vector.tensor_scalar_add(st[:, :w], ps[:, :w],
                                                bias_sb[:, m:m + 1])
                    nc.sync.dma_start(
                        xg_dram[:, m, c0 // B:(c0 + w) // B, :]
                        .rearrange("p t b -> p (t b)"),
                        st[:, :w])

                for c in (0, 1):
                    for m in range(4):
                        emit_a(c, m)

                hsT = [None] * NT
                ring = [None] * NT
                xg_tiles = {}

                def prefetch_xg(tp):
                    xg_sb = xgt.tile([128, 4, B], dt.float16, tag="xgt",
                                     name=f"xg{rep}_{tp}")
                    nc.sync.dma_start(xg_sb[:], xg_dram[:, :, tp, :])
                    xg_tiles[tp] = xg_sb

                def emit_c1(j, n):
                    ps = psC.tile([128, VCH], dt.float32, tag="psC",
                                  name=f"psc{rep}_{j}_{n}")
                    for k in range(KT):
                        nc.tensor.matmul(
                            ps[:],
                            hsT[j][:, k, :, :].rearrange("p t b -> p (t b)"),
                            wout_sb[:, k, n * VCH:(n + 1) * VCH],
                            start=(k == 0), stop=(k == KT - 1))
                    if n == 0:
                        ring[j] = p8k.tile([128, VPC], dt.float16, tag="p8k",
                                           name=f"ring{rep}_{j}")
                    sl = ring[j][:, n * VCH:(n + 1) * VCH]
                    nc.vector.tensor_add(sl, ps[:],
                                         bout_sb[:, n * VCH:(n + 1) * VCH])
                    if n == NCH - 1:
                        scr = c1p.tile([128, VPC], dt.float16, tag="scr",
                                       name=f"scr{rep}_{j}")
                        nc.scalar.activation(scr[:], ring[j][:], AF.Exp,
                                             accum_out=s_st[:, j:j + 1])

                def emit_corr(j, cc_out, col):
                    srec = tiny.tile([128, NC, 1], dt.float16, tag="srec",
                                     name=f"srec{rep}_{j}")
                    nc.sync.dma_start(
                        srec[:],
                        cc_out[:, col:col + 1].rearrange("(r p) c -> p r c", p=128))
                    stot = tiny.tile([128, 1], dt.float32, tag="stot",
                                     name=f"stot{rep}_{j}")
                    nc.vector.tensor_reduce(stot[:],
                                            srec[:].rearrange("p r c -> p (r c)"),
                                            axis=AX.X, op=ALU.add)
                    nc.scalar.activation(corr[:, j:j + 1], stot[:], AF.Ln)

                def emit_c2(j, n):
                    neg = tiny.tile([128, 1], dt.float32, tag="neg",
                                    name=f"neg{rep}_{j}_{n}")
                    nc.vector.tensor_scalar_mul(neg[:], corr[:, j:j + 1], -1.0)
                    so = c2p.tile([128, C2W], dt.float32, tag="c2o",
                                  name=f"c2o{rep}_{j}_{n}")
                    nc.vector.tensor_scalar_add(
                        so[:], ring[j][:, n * C2W:(n + 1) * C2W], neg[:])
                    nc.sync.dma_start(
                        out_logp.rearrange("b t v -> t b v")
                        [2 * j:2 * j + 2, :, n * C2W:(n + 1) * C2W],
                        so[:])

                # ---- phase B with interleaved A/C ----
                for t in range(T):
                    if t == 0:
                        for tp in (0, 1, 2):
                            prefetch_xg(tp)
                        for k in range(4):
                            load_wout(k)
                    elif t == 1:
                        for k in range(4, KT):
                            load_wout(k)
                    if t > 0 and t + 2 < T:
                        prefetch_xg(t + 2)
                    gp = psB.tile([128, 4 * B], dt.float32, tag="psB",
                                  name=f"gp{rep}_{t}")
                    for m in range(4):
                        for k in range(KT):
                            if t == 0:
                                rhs = h0_sb[:, k, :]
                            else:
                                pj, tau = divmod(t - 1, 2)
                                rhs = hsT[pj][:, k, tau, :]
                            nc.tensor.matmul(
                                gp[:, m * B:(m + 1) * B],
                                whh_sb[:, k, m * 128:(m + 1) * 128],
                                rhs, start=(k == 0), stop=(k == KT - 1))
                    gs = bwork.tile([128, 4 * B], dt.float32, tag="gs",
                                    name=f"gs{rep}_{t}")
                    nc.vector.tensor_add(gs[:], gp[:],
                                         xg_tiles[t][:].rearrange("p g b -> p (g b)"))
                    act = bwork.tile([128, 4 * B], dt.float32, tag="act",
                                     name=f"act{rep}_{t}")
                    nc.scalar.activation(act[:, 0:3 * B], gs[:, 0:3 * B], AF.Sigmoid)
                    nc.scalar.activation(act[:, 3 * B:4 * B], gs[:, 3 * B:4 * B],
                                         AF.Sigmoid, scale=2.0)
                    gt = bwork.tile([128, B], dt.float32, tag="gt", name=f"gt{rep}_{t}")
                    nc.vector.tensor_scalar(gt[:], act[:, 3 * B:4 * B], 2.0, -1.0,
                                            op0=ALU.mult, op1=ALU.add)
                    ig = bwork.tile([128, B], dt.float32, tag="ig", name=f"ig{rep}_{t}")
                    nc.vector.tensor_mul(ig[:], act[:, 0:B], gt[:])
                    fc = bwork.tile([128, B], dt.float32, tag="fc", name=f"fc{rep}_{t}")
                    nc.vector.tensor_mul(fc[:], act[:, B:2 * B], c_sb[:])
                    nc.vector.tensor_add(c_sb[:], fc[:], ig[:])
                    tcs = bwork.tile([128, B], dt.float32, tag="tcs",
                                     name=f"tcs{rep}_{t}")
                    nc.scalar.activation(tcs[:], c_sb[:], AF.Sigmoid, scale=2.0)
                    tcc = bwork.tile([128, B], dt.float32, tag="tcc",
                                     name=f"tcc{rep}_{t}")
                    nc.vector.tensor_scalar(tcc[:], tcs[:], 2.0, -1.0,
                                            op0=ALU.mult, op1=ALU.add)

                    att = plan[t]["attach"]
                    w = 65 if att is not None else 64
                    pay = payp.tile([128, w], dt.float16, tag=f"pay{w}",
                                    name=f"pay{rep}_{t}")
                    nc.vector.tensor_mul(pay[:, 0:B], act[:, 2 * B:3 * B], tcc[:])

                    if t == T - 1:
                        h_t = bwork.tile([128, B], dt.float32, tag="ht",
                                         name=f"ht{rep}_{t}")
                        nc.vector.tensor_mul(h_t[:], act[:, 2 * B:3 * B], tcc[:])
                        nc.sync.dma_start(out_hT, h_t[:])
                        nc.sync.dma_start(out_cT, c_sb[:])
                    if att is not None:
                        nc.vector.tensor_copy(pay[:, 64:65], s_st[:, att:att + 1])
                    cc_in = dram.tile([128, w], dt.float16, tag=f"cin{w}",
                                      name=f"ccin{rep}_{t}")
                    nc.sync.dma_start(cc_in[:], pay[:])
                    cc_out = dramo.tile([NC * 128, w], dt.float16, tag=f"cout{w}",
                                        name=f"ccout{rep}_{t}", addr_space="Shared")
                    if use_cc:
                        nc.gpsimd.collective_compute(
                            "AllGather", mybir.AluOpType.bypass,
                            ins=[cc_in[:].opt()], outs=[cc_out[:].opt()],
                            replica_groups=rg)
                        recv_src = cc_out[:, 0:B].rearrange("(r p) b -> p r b", p=128)
                    else:
                        recv_src = cc_in[:, 0:B].rearrange(
                            "p (x b) -> p x b", x=1).broadcast_to([128, NC, B])
                    j, tau = divmod(t, 2)
                    if tau == 0:
                        hsT[j] = hst.tile([128, KT, 2, B], dt.float16, tag="hsT",
                                          name=f"hsT{rep}_{j}")
                    nc.sync.dma_start(hsT[j][:, :, tau, :], recv_src)
                    for it in plan[t]["pe"]:
                        if it[0] == "A":
                            emit_a(it[1], it[2])
                        else:
                            emit_c1(it[1], it[2])
                    if att is not None:
                        if use_cc:
                            emit_corr(att, cc_out, 64)
                        else:
                            nc.vector.tensor_copy(corr[:, att:att + 1],
                                                  s_st[:, att:att + 1])
                    for (cj, cn) in plan[t]["c2"]:
                        emit_c2(cj, cn)

                # ---- tail ----
                for (cj, cn) in tail["c1"]:
                    emit_c1(cj, cn)
                ntail = len(tail["corr"])
                if ntail and not use_cc:
                    for j in tail["corr"]:
                        nc.vector.tensor_copy(corr[:, j:j + 1], s_st[:, j:j + 1])
                if ntail and use_cc:
                    payt = payp.tile([128, ntail], dt.float16, tag="payt",
                                     name=f"payt{rep}")
                    for i, j in enumerate(tail["corr"]):
                        nc.vector.tensor_copy(payt[:, i:i + 1], s_st[:, j:j + 1])
                    cc_in = dram.tile([128, ntail], dt.float16, tag="cint",
                                      name=f"ccint{rep}")
                    nc.sync.dma_start(cc_in[:], payt[:])
                    cc_out = dramo.tile([NC * 128, ntail], dt.float16, tag="coutt",
                                        name=f"ccoutt{rep}", addr_space="Shared")
                    nc.gpsimd.collective_compute(
                        "AllGather", mybir.AluOpType.bypass,
                        ins=[cc_in[:].opt()], outs=[cc_out[:].opt()],
                        replica_groups=rg)
                    for i, j in enumerate(tail["corr"]):
                        emit_corr(j, cc_out, i)
                for (cj, cn) in tail["c2"]:
                    emit_c2(cj, cn)

    nc.compile()
    return nc


def _prep(inputs):
    f16 = np.float16
    emb = inputs["emb"]
    tgt = inputs["target_tensor"]
    tokens = np.concatenate(
        [np.full((B, 1), SOS, tgt.dtype), tgt[:, :T - 1]], axis=1)
    x = np.maximum(emb[tokens], 0.0)
    xT16 = np.ascontiguousarray(x.transpose(2, 1, 0).reshape(H, T * B)).astype(f16)
    bsum = (inputs["b_ih"] + inputs["b_hh"]).astype(np.float32)
    W_ih, W_hh = inputs["W_ih"], inputs["W_hh"]
    W_out, b_out = inputs["W_out"], inputs["b_out"]
    h0 = inputs["encoder_hidden_h"][0]
    c0 = inputs["encoder_hidden_c"][0]
    h0T16 = np.ascontiguousarray(h0.T).astype(f16)
    c0T = np.ascontiguousarray(c0.T).astype(np.float32)

    in_maps = []
    for k in range(NC):
        rows = np.concatenate([o * H + np.arange(k * HPC, (k + 1) * HPC)
                               for o in (0, 1, 3, 2)])
        in_maps.append({
            "xT": xT16,
            "wihT": np.ascontiguousarray(W_ih[rows].T).astype(f16),
            "whhT": np.ascontiguousarray(W_hh[rows].T).astype(f16),
            "bias4": np.ascontiguousarray(bsum[rows].reshape(4, HPC).T),
            "woutT": np.ascontiguousarray(W_out[k * VPC:(k + 1) * VPC].T).astype(f16),
            "boutr": b_out[k * VPC:(k + 1) * VPC].reshape(1, VPC).astype(f16),
            "h0T": h0T16,
            "c0T": np.ascontiguousarray(c0T[k * HPC:(k + 1) * HPC]),
        })
    return in_maps


def kernel(**inputs):
    global _compiled
    from concourse.bass_utils import run_bass_kernel_spmd
    inputs = {k: np.asarray(v) for k, v in inputs.items()}
    if _compiled is None:
        _compiled = _build()
    in_maps = _prep(inputs)
    res = run_bass_kernel_spmd(_compiled, in_maps, list(range(NC)))
    r = res.results
    logp = np.concatenate([r[k]["out_logp"] for k in range(NC)], axis=2)
    hT = np.zeros((1, B, H), np.float32)
    cT = np.zeros((1, B, H), np.float32)
    for k in range(NC):
        hT[0, :, k * HPC:(k + 1) * HPC] = r[k]["out_hT"].T
        cT[0, :, k * HPC:(k + 1) * HPC] = r[k]["out_cT"].T
    return logp, hT, cT
